# revision 24
# baseline (speedup 1.0000x reference)
"""Voronoi-region sparse attention for Trainium2, 8-core SPMD. (v1' bisect)"""
import sys
import os

sys.path.insert(0, "/opt/trn_rl_repo")

import numpy as np
import ml_dtypes

B, N, C, H = 2, 65536, 96, 3
HD = C // H
R, S = 256, 256
NCORES = 8
T = (B * N) // NCORES
RPC = T // S
CHUNK_REGIONS = 8
CHUNK_T = CHUNK_REGIONS * S
NCHUNKS = RPC // CHUNK_REGIONS
SCALE = float(HD) ** -0.5

_STATE = {}
_PROFILE_DIR = None

PIPELINE = True          # bisect knob: lag-1 software pipelining


def _build_nc():
    import concourse.bacc as bacc
    import concourse.mybir as mybir
    import concourse.tile as tile

    dt = mybir.dt
    F32, BF16, F32R = dt.float32, dt.bfloat16, dt.float32r
    AF = mybir.ActivationFunctionType
    add = mybir.AluOpType.add
    mult = mybir.AluOpType.mult

    nc = bacc.Bacc("TRN2", target_bir_lowering=False, debug=False,
                   num_devices=NCORES)

    xq_d = nc.dram_tensor("xq_t", [C, T], BF16, kind="ExternalInput")
    xk_d = nc.dram_tensor("xk_t", [C, T], BF16, kind="ExternalInput")
    xv_d = nc.dram_tensor("xv_t", [C, T], BF16, kind="ExternalInput")
    wq_d = nc.dram_tensor("wq_b", [C, C], BF16, kind="ExternalInput")
    wp_d = nc.dram_tensor("wp", [C, C], F32, kind="ExternalInput")
    bq_d = nc.dram_tensor("bq", [C, 1], F32, kind="ExternalInput")
    bp_d = nc.dram_tensor("bp", [C, 1], F32, kind="ExternalInput")
    id_d = nc.dram_tensor("ident", [C, C], BF16, kind="ExternalInput")
    out_d = nc.dram_tensor("out_t", [C, T], F32, kind="ExternalOutput")

    with tile.TileContext(nc) as tc:
        with (
            tc.tile_pool(name="const", bufs=1) as cpool,
            tc.tile_pool(name="xin", bufs=2) as xin_pool,
            tc.tile_pool(name="qkv", bufs=2) as qkv_pool,
            tc.tile_pool(name="vtok", bufs=2) as v_pool,
            tc.tile_pool(name="p", bufs=4) as p_pool,
            tc.tile_pool(name="recip", bufs=2) as recip_pool,
            tc.tile_pool(name="onorm", bufs=2) as onorm_pool,
            tc.tile_pool(name="outsb", bufs=2) as out_pool,
            tc.tile_pool(name="proj_ps", bufs=1, space="PSUM") as proj_psum,
            tc.tile_pool(name="score_ps", bufs=1, space="PSUM") as score_psum,
            tc.tile_pool(name="pv_ps", bufs=2, space="PSUM") as pv_psum,
            tc.tile_pool(name="vtr_ps", bufs=1, space="PSUM") as vtr_psum,
        ):
            wq = cpool.tile([C, C], BF16)
            nc.sync.dma_start(wq[:], wq_d[:])
            wp = cpool.tile([C, C], F32)
            nc.sync.dma_start(wp[:], wp_d[:])
            wp_r = cpool.tile([C, C], F32R)
            nc.vector.tensor_copy(wp_r[:], wp[:])
            bq = cpool.tile([C, 1], F32)
            nc.sync.dma_start(bq[:], bq_d[:])
            bp = cpool.tile([C, 1], F32)
            nc.sync.dma_start(bp[:], bp_d[:])
            ident = cpool.tile([C, C], BF16)
            nc.sync.dma_start(ident[:], id_d[:])
            ones32 = cpool.tile([128, HD], BF16)
            nc.vector.memset(ones32[:], 1.0)

            chunks = {}

            def chunk_alloc(ck):
                t0 = ck * CHUNK_T
                xq = xin_pool.tile([C, CHUNK_T], BF16, tag="xq", name="xq")
                nc.sync.dma_start(xq[:], xq_d[:, t0:t0 + CHUNK_T])
                xk = xin_pool.tile([C, CHUNK_T], BF16, tag="xk", name="xk")
                nc.sync.dma_start(xk[:], xk_d[:, t0:t0 + CHUNK_T])
                xv = xin_pool.tile([C, CHUNK_T], BF16, tag="xv", name="xv")
                nc.sync.dma_start(xv[:], xv_d[:, t0:t0 + CHUNK_T])
                chunks[ck] = {
                    "xq": xq, "xk": xk, "xv": xv,
                    "qt": qkv_pool.tile([C, CHUNK_T], BF16, tag="qt",
                                        name="qt"),
                    "kt": qkv_pool.tile([C, CHUNK_T], BF16, tag="kt",
                                        name="kt"),
                    "vt": qkv_pool.tile([C, CHUNK_T], BF16, tag="vt",
                                        name="vt"),
                    "v_sb": v_pool.tile([128, 2 * CHUNK_REGIONS, C], BF16,
                                        name="v_sb"),
                    "o_norm": onorm_pool.tile([C, CHUNK_T], F32R,
                                              name="o_norm"),
                    "out_sb": out_pool.tile([C, CHUNK_T], F32,
                                            name="out_sb"),
                }

            def emit_proj(ck, which, s2):
                ch = chunks[ck]
                src, dst = ch["x" + which[0]], ch[which]
                o = s2 * 1024
                ps = proj_psum.tile([C, 2, 512], F32, tag="proj", name="ps")
                nc.tensor.matmul(ps[:, 0, :], wq[:], src[:, o:o + 512],
                                 start=True, stop=True)
                nc.tensor.matmul(ps[:, 1, :], wq[:], src[:, o + 512:o + 1024],
                                 start=True, stop=True)
                flat = ps[:].rearrange("p a b -> p (a b)")
                nc.vector.tensor_scalar(
                    out=dst[:, o:o + 768], in0=flat[:, 0:768],
                    scalar1=bq[:], scalar2=None, op0=add)
                nc.scalar.activation(
                    dst[:, o + 768:o + 1024], flat[:, 768:1024],
                    AF.Identity, bias=bq[:], scale=1.0)

            def emit_vtrans(ck, q):
                vt, v_sb = chunks[ck]["vt"], chunks[ck]["v_sb"]
                ps = vtr_psum.tile([128, 4, 128], BF16, tag="vtr", name="vtr")
                for jj in range(4):
                    j = q * 4 + jj
                    nc.tensor.transpose(ps[:, jj, 0:C],
                                        vt[:, j * 128:(j + 1) * 128],
                                        ident[:])
                nc.vector.tensor_copy(v_sb[:, q * 4:q * 4 + 4, :],
                                      ps[:, :, 0:C])

            def emit_scores_exp(r):
                ch = chunks[r // CHUNK_REGIONS]
                qt, kt = ch["qt"], ch["kt"]
                r0 = (r % CHUNK_REGIONS) * S
                s_ps = score_psum.tile([128, 6, S], F32, tag="scores",
                                       name="s_ps")
                for half in range(2):
                    for h in range(H):
                        nc.tensor.matmul(
                            s_ps[:, h * 2 + half, :],
                            kt[HD * h:HD * (h + 1),
                               r0 + 128 * half:r0 + 128 * (half + 1)],
                            qt[HD * h:HD * (h + 1), r0:r0 + S],
                            start=True, stop=True)
                p_sb = p_pool.tile([128, 6, S], BF16, name="p_sb")
                nc.scalar.activation(p_sb[:], s_ps[:], AF.Exp, scale=SCALE)
                return p_sb

            def emit_pv(r, p_sb, pvod):
                v_sb = chunks[r // CHUNK_REGIONS]["v_sb"]
                rl = r % CHUNK_REGIONS
                for h in range(H):
                    for half in range(2):
                        nc.tensor.matmul(
                            pvod[HD * h:HD * (h + 1), 0, :],
                            v_sb[:, 2 * rl + half, HD * h:HD * (h + 1)],
                            p_sb[:, h * 2 + half, :],
                            start=(half == 0), stop=(half == 1),
                            tile_position=(0, HD * h))
                for h in range(H):
                    for half in range(2):
                        nc.tensor.matmul(
                            pvod[HD * h:HD * (h + 1), 1, :],
                            ones32[:],
                            p_sb[:, h * 2 + half, :],
                            start=(half == 0), stop=(half == 1),
                            tile_position=(0, HD * h))

            def emit_norm(r, pvod):
                rl = r % CHUNK_REGIONS
                o_norm = chunks[r // CHUNK_REGIONS]["o_norm"]
                recip = recip_pool.tile([C, S], F32, name="recip")
                nc.vector.reciprocal_approx_fast(out=recip[:],
                                                 in_=pvod[:, 1, :])
                nc.vector.tensor_tensor(
                    out=o_norm[:, rl * S:(rl + 1) * S],
                    in0=pvod[:, 0, :],
                    in1=recip[:],
                    op=mult)

            def emit_outproj(ck, s2):
                ch = chunks[ck]
                o = s2 * 1024
                ps = proj_psum.tile([C, 2, 512], F32, tag="proj", name="ps")
                nc.tensor.matmul(ps[:, 0, :], wp_r[:],
                                 ch["o_norm"][:, o:o + 512],
                                 start=True, stop=True)
                nc.tensor.matmul(ps[:, 1, :], wp_r[:],
                                 ch["o_norm"][:, o + 512:o + 1024],
                                 start=True, stop=True)
                flat = ps[:].rearrange("p a b -> p (a b)")
                nc.vector.tensor_scalar(
                    out=ch["out_sb"][:, o:o + 768], in0=flat[:, 0:768],
                    scalar1=bp[:], scalar2=None, op0=add)
                nc.scalar.activation(
                    ch["out_sb"][:, o + 768:o + 1024], flat[:, 768:1024],
                    AF.Identity, bias=bp[:], scale=1.0)

            def chunk_finish(ck):
                for s2 in range(2):
                    emit_outproj(ck, s2)
                t0 = ck * CHUNK_T
                nc.sync.dma_start(out_d[:, t0:t0 + CHUNK_T],
                                  chunks[ck]["out_sb"][:])
                del chunks[ck]

            prev = None
            o_ps = den_ps = None
            lag = 1 if PIPELINE else 0
            for i in range(RPC + lag):
                ck, r = divmod(i, CHUNK_REGIONS)
                if i < RPC:
                    if r == 0:
                        chunk_alloc(ck)
                        for s2 in range(2):
                            for which in ("qt", "kt", "vt"):
                                emit_proj(ck, which, s2)
                        for q in range(4):
                            emit_vtrans(ck, q)
                    pvod = pv_psum.tile([C, 2, S], F32, tag="pvod",
                                        name="pvod")
                    p_sb = emit_scores_exp(i)
                    cur = (i, p_sb, pvod, pvod)
                else:
                    cur = None

                if not PIPELINE:
                    prev, cur = cur, None
                if prev is not None:
                    pr, pp, po, pd = prev
                    emit_pv(pr, pp, po)
                    emit_norm(pr, po)
                    if pr % CHUNK_REGIONS == CHUNK_REGIONS - 1:
                        chunk_finish(pr // CHUNK_REGIONS)
                if PIPELINE:
                    prev = cur

    nc.compile()
    return nc


def _get_nc():
    if "nc" not in _STATE:
        _STATE["nc"] = _build_nc()
    return _STATE["nc"]


def kernel(xq, xk, xv, Wq, bq, Wp, bp, Voronoi):
    from concourse.bass_utils import run_bass_kernel_spmd

    bf16 = ml_dtypes.bfloat16
    xq = np.asarray(xq, np.float32)
    xk = np.asarray(xk, np.float32)
    xv = np.asarray(xv, np.float32)
    Wq = np.asarray(Wq, np.float32)
    Wp = np.asarray(Wp, np.float32)
    bq = np.asarray(bq, np.float32)
    bp = np.asarray(bp, np.float32)

    perms = [np.argsort(np.asarray(Voronoi[b]).reshape(-1), kind="stable")
             for b in range(B)]

    wq_b = Wq.astype(bf16)
    wp_f = np.ascontiguousarray(Wp)
    bq_c = np.ascontiguousarray(bq.reshape(C, 1))
    bp_c = np.ascontiguousarray(bp.reshape(C, 1))
    ident = np.eye(C, dtype=bf16)

    in_maps = []
    for core in range(NCORES):
        b, g = divmod(core, NCORES // B)
        idx = perms[b][g * T:(g + 1) * T]
        in_maps.append({
            "xq_t": np.ascontiguousarray(xq[b][idx].T).astype(bf16),
            "xk_t": np.ascontiguousarray(xk[b][idx].T).astype(bf16),
            "xv_t": np.ascontiguousarray(xv[b][idx].T).astype(bf16),
            "wq_b": wq_b, "wp": wp_f, "bq": bq_c, "bp": bp_c,
            "ident": ident,
        })

    nc = _get_nc()
    if _PROFILE_DIR:
        run_bass_kernel_spmd(nc, in_maps, core_ids=list(range(NCORES)))
        from trn_agent_boot.trn_boot import _ntff_profile_via_ctypes
        from concourse import bass2jax
        hook = _ntff_profile_via_ctypes("/opt/axon/libaxon_pjrt.so")
        os.makedirs(_PROFILE_DIR, exist_ok=True)
        with hook(_PROFILE_DIR, list(range(NCORES))):
            results = bass2jax.run_bass_via_pjrt(nc, in_maps,
                                                 n_cores=NCORES)
    else:
        results = run_bass_kernel_spmd(
            nc, in_maps, core_ids=list(range(NCORES))).results

    out = np.empty((B, N, C), np.float32)
    for core in range(NCORES):
        b, g = divmod(core, NCORES // B)
        idx = perms[b][g * T:(g + 1) * T]
        out[b][idx] = results[core]["out_t"].T
    return out


# revision 25
# speedup vs baseline: 1.3747x; 1.3747x over previous
"""Voronoi-region sparse attention for Trainium2, 8-core SPMD. (v1' bisect)"""
import sys
import os

sys.path.insert(0, "/opt/trn_rl_repo")

import numpy as np
import ml_dtypes

B, N, C, H = 2, 65536, 96, 3
HD = C // H
R, S = 256, 256
NCORES = 8
T = (B * N) // NCORES
RPC = T // S
CHUNK_REGIONS = 8
CHUNK_T = CHUNK_REGIONS * S
NCHUNKS = RPC // CHUNK_REGIONS
SCALE = float(HD) ** -0.5

_STATE = {}
_PROFILE_DIR = None

PIPELINE = True          # bisect knob: lag-1 software pipelining


def _build_nc():
    import concourse.bacc as bacc
    import concourse.mybir as mybir
    import concourse.tile as tile

    dt = mybir.dt
    F32, BF16, F32R = dt.float32, dt.bfloat16, dt.float32r
    AF = mybir.ActivationFunctionType
    add = mybir.AluOpType.add
    mult = mybir.AluOpType.mult

    nc = bacc.Bacc("TRN2", target_bir_lowering=False, debug=False,
                   num_devices=NCORES)

    xq_d = nc.dram_tensor("xq_t", [C, T], BF16, kind="ExternalInput")
    xk_d = nc.dram_tensor("xk_t", [C, T], BF16, kind="ExternalInput")
    xv_d = nc.dram_tensor("xv_t", [C, T], BF16, kind="ExternalInput")
    wq_d = nc.dram_tensor("wq_b", [C, C], BF16, kind="ExternalInput")
    wp_d = nc.dram_tensor("wp", [C, C], F32, kind="ExternalInput")
    bq_d = nc.dram_tensor("bq", [C, 1], F32, kind="ExternalInput")
    bp_d = nc.dram_tensor("bp", [C, 1], F32, kind="ExternalInput")
    id_d = nc.dram_tensor("ident", [C, C], BF16, kind="ExternalInput")
    out_d = nc.dram_tensor("out_t", [C, T], F32, kind="ExternalOutput")

    with tile.TileContext(nc) as tc:
        with (
            tc.tile_pool(name="const", bufs=1) as cpool,
            tc.tile_pool(name="xin", bufs=2) as xin_pool,
            tc.tile_pool(name="qkv", bufs=2) as qkv_pool,
            tc.tile_pool(name="vtok", bufs=2) as v_pool,
            tc.tile_pool(name="p", bufs=4) as p_pool,
            tc.tile_pool(name="recip", bufs=2) as recip_pool,
            tc.tile_pool(name="onorm", bufs=2) as onorm_pool,
            tc.tile_pool(name="outsb", bufs=2) as out_pool,
            tc.tile_pool(name="proj_ps", bufs=1, space="PSUM") as proj_psum,
            tc.tile_pool(name="score_ps", bufs=1, space="PSUM") as score_psum,
            tc.tile_pool(name="pv_ps", bufs=2, space="PSUM") as pv_psum,
            tc.tile_pool(name="vtr_ps", bufs=1, space="PSUM") as vtr_psum,
        ):
            wq = cpool.tile([C, C], BF16)
            nc.sync.dma_start(wq[:], wq_d[:])
            wp = cpool.tile([C, C], F32)
            nc.sync.dma_start(wp[:], wp_d[:])
            wp_r = cpool.tile([C, C], F32R)
            nc.vector.tensor_copy(wp_r[:], wp[:])
            bq = cpool.tile([C, 1], F32)
            nc.sync.dma_start(bq[:], bq_d[:])
            bp = cpool.tile([C, 1], F32)
            nc.sync.dma_start(bp[:], bp_d[:])
            ident = cpool.tile([C, C], BF16)
            nc.sync.dma_start(ident[:], id_d[:])
            ones32 = cpool.tile([128, HD], BF16)
            nc.vector.memset(ones32[:], 1.0)

            chunks = {}

            def chunk_alloc(ck):
                t0 = ck * CHUNK_T
                xq = xin_pool.tile([C, CHUNK_T], BF16, tag="xq", name="xq")
                nc.sync.dma_start(xq[:], xq_d[:, t0:t0 + CHUNK_T])
                xk = xin_pool.tile([C, CHUNK_T], BF16, tag="xk", name="xk")
                nc.sync.dma_start(xk[:], xk_d[:, t0:t0 + CHUNK_T])
                xv = xin_pool.tile([C, CHUNK_T], BF16, tag="xv", name="xv")
                nc.sync.dma_start(xv[:], xv_d[:, t0:t0 + CHUNK_T])
                chunks[ck] = {
                    "xq": xq, "xk": xk, "xv": xv,
                    "qt": qkv_pool.tile([C, CHUNK_T], BF16, tag="qt",
                                        name="qt"),
                    "kt": qkv_pool.tile([C, CHUNK_T], BF16, tag="kt",
                                        name="kt"),
                    "vt": qkv_pool.tile([C, CHUNK_T], BF16, tag="vt",
                                        name="vt"),
                    "v_sb": v_pool.tile([128, 2 * CHUNK_REGIONS, C], BF16,
                                        name="v_sb"),
                    "o_norm": onorm_pool.tile([C, CHUNK_T], F32R,
                                              name="o_norm"),
                    "out_sb": out_pool.tile([C, CHUNK_T], F32,
                                            name="out_sb"),
                }

            def emit_proj(ck, which, s2):
                ch = chunks[ck]
                src, dst = ch["x" + which[0]], ch[which]
                o = s2 * 1024
                ps = proj_psum.tile([C, 2, 512], F32, tag="proj", name="ps")
                nc.tensor.matmul(ps[:, 0, :], wq[:], src[:, o:o + 512],
                                 start=True, stop=True)
                nc.tensor.matmul(ps[:, 1, :], wq[:], src[:, o + 512:o + 1024],
                                 start=True, stop=True)
                nc.vector.tensor_scalar(
                    out=dst[:, o:o + 1024],
                    in0=ps[:].rearrange("p a b -> p (a b)"),
                    scalar1=bq[:], scalar2=None, op0=add)

            def emit_vtrans(ck, q):
                vt, v_sb = chunks[ck]["vt"], chunks[ck]["v_sb"]
                ps = vtr_psum.tile([128, 4, 128], BF16, tag="vtr", name="vtr")
                for jj in range(4):
                    j = q * 4 + jj
                    nc.tensor.transpose(ps[:, jj, 0:C],
                                        vt[:, j * 128:(j + 1) * 128],
                                        ident[:])
                nc.vector.tensor_copy(v_sb[:, q * 4:q * 4 + 4, :],
                                      ps[:, :, 0:C])

            def emit_scores_exp(r):
                ch = chunks[r // CHUNK_REGIONS]
                qt, kt = ch["qt"], ch["kt"]
                r0 = (r % CHUNK_REGIONS) * S
                s_ps = score_psum.tile([128, 6, S], F32, tag="scores",
                                       name="s_ps")
                for half in range(2):
                    for h in range(H):
                        nc.tensor.matmul(
                            s_ps[:, h * 2 + half, :],
                            kt[HD * h:HD * (h + 1),
                               r0 + 128 * half:r0 + 128 * (half + 1)],
                            qt[HD * h:HD * (h + 1), r0:r0 + S],
                            start=True, stop=True)
                p_sb = p_pool.tile([128, 6, S], BF16, name="p_sb")
                nc.scalar.activation(p_sb[:], s_ps[:], AF.Exp, scale=SCALE)
                return p_sb

            def emit_pv(r, p_sb, pvod):
                v_sb = chunks[r // CHUNK_REGIONS]["v_sb"]
                rl = r % CHUNK_REGIONS
                for h in range(H):
                    for half in range(2):
                        nc.tensor.matmul(
                            pvod[HD * h:HD * (h + 1), 0, :],
                            v_sb[:, 2 * rl + half, HD * h:HD * (h + 1)],
                            p_sb[:, h * 2 + half, :],
                            start=(half == 0), stop=(half == 1),
                            tile_position=(0, HD * h))
                for h in range(H):
                    for half in range(2):
                        nc.tensor.matmul(
                            pvod[HD * h:HD * (h + 1), 1, :],
                            ones32[:],
                            p_sb[:, h * 2 + half, :],
                            start=(half == 0), stop=(half == 1),
                            tile_position=(0, HD * h))

            def emit_norm(r, pvod):
                rl = r % CHUNK_REGIONS
                o_norm = chunks[r // CHUNK_REGIONS]["o_norm"]
                recip = recip_pool.tile([C, S], F32, name="recip")
                nc.vector.reciprocal_approx_fast(out=recip[:],
                                                 in_=pvod[:, 1, :])
                nc.vector.tensor_tensor(
                    out=o_norm[:, rl * S:(rl + 1) * S],
                    in0=pvod[:, 0, :],
                    in1=recip[:],
                    op=mult)

            def emit_outproj(ck, s2):
                ch = chunks[ck]
                o = s2 * 1024
                ps = proj_psum.tile([C, 2, 512], F32, tag="proj", name="ps")
                nc.tensor.matmul(ps[:, 0, :], wp_r[:],
                                 ch["o_norm"][:, o:o + 512],
                                 start=True, stop=True)
                nc.tensor.matmul(ps[:, 1, :], wp_r[:],
                                 ch["o_norm"][:, o + 512:o + 1024],
                                 start=True, stop=True)
                nc.vector.tensor_scalar(
                    out=ch["out_sb"][:, o:o + 1024],
                    in0=ps[:].rearrange("p a b -> p (a b)"),
                    scalar1=bp[:], scalar2=None, op0=add)

            def chunk_finish(ck):
                for s2 in range(2):
                    emit_outproj(ck, s2)
                t0 = ck * CHUNK_T
                nc.sync.dma_start(out_d[:, t0:t0 + CHUNK_T],
                                  chunks[ck]["out_sb"][:])
                del chunks[ck]

            prev = None
            o_ps = den_ps = None
            lag = 1 if PIPELINE else 0
            for i in range(RPC + lag):
                ck, r = divmod(i, CHUNK_REGIONS)
                if i < RPC:
                    if r == 0:
                        chunk_alloc(ck)
                        for s2 in range(2):
                            for which in ("qt", "kt", "vt"):
                                emit_proj(ck, which, s2)
                        for q in range(4):
                            emit_vtrans(ck, q)
                    pvod = pv_psum.tile([C, 2, S], F32, tag="pvod",
                                        name="pvod")
                    p_sb = emit_scores_exp(i)
                    cur = (i, p_sb, pvod, pvod)
                else:
                    cur = None

                if not PIPELINE:
                    prev, cur = cur, None
                if prev is not None:
                    pr, pp, po, pd = prev
                    emit_pv(pr, pp, po)
                    emit_norm(pr, po)
                    if pr % CHUNK_REGIONS == CHUNK_REGIONS - 1:
                        chunk_finish(pr // CHUNK_REGIONS)
                if PIPELINE:
                    prev = cur

    nc.compile()
    return nc


def _get_nc():
    if "nc" not in _STATE:
        _STATE["nc"] = _build_nc()
    return _STATE["nc"]


def kernel(xq, xk, xv, Wq, bq, Wp, bp, Voronoi):
    from concourse.bass_utils import run_bass_kernel_spmd

    bf16 = ml_dtypes.bfloat16
    xq = np.asarray(xq, np.float32)
    xk = np.asarray(xk, np.float32)
    xv = np.asarray(xv, np.float32)
    Wq = np.asarray(Wq, np.float32)
    Wp = np.asarray(Wp, np.float32)
    bq = np.asarray(bq, np.float32)
    bp = np.asarray(bp, np.float32)

    perms = [np.argsort(np.asarray(Voronoi[b]).reshape(-1), kind="stable")
             for b in range(B)]

    wq_b = Wq.astype(bf16)
    wp_f = np.ascontiguousarray(Wp)
    bq_c = np.ascontiguousarray(bq.reshape(C, 1))
    bp_c = np.ascontiguousarray(bp.reshape(C, 1))
    ident = np.eye(C, dtype=bf16)

    in_maps = []
    for core in range(NCORES):
        b, g = divmod(core, NCORES // B)
        idx = perms[b][g * T:(g + 1) * T]
        in_maps.append({
            "xq_t": np.ascontiguousarray(xq[b][idx].T).astype(bf16),
            "xk_t": np.ascontiguousarray(xk[b][idx].T).astype(bf16),
            "xv_t": np.ascontiguousarray(xv[b][idx].T).astype(bf16),
            "wq_b": wq_b, "wp": wp_f, "bq": bq_c, "bp": bp_c,
            "ident": ident,
        })

    nc = _get_nc()
    if _PROFILE_DIR:
        run_bass_kernel_spmd(nc, in_maps, core_ids=list(range(NCORES)))
        from trn_agent_boot.trn_boot import _ntff_profile_via_ctypes
        from concourse import bass2jax
        hook = _ntff_profile_via_ctypes("/opt/axon/libaxon_pjrt.so")
        os.makedirs(_PROFILE_DIR, exist_ok=True)
        with hook(_PROFILE_DIR, list(range(NCORES))):
            results = bass2jax.run_bass_via_pjrt(nc, in_maps,
                                                 n_cores=NCORES)
    else:
        results = run_bass_kernel_spmd(
            nc, in_maps, core_ids=list(range(NCORES))).results

    out = np.empty((B, N, C), np.float32)
    for core in range(NCORES):
        b, g = divmod(core, NCORES // B)
        idx = perms[b][g * T:(g + 1) * T]
        out[b][idx] = results[core]["out_t"].T
    return out
